# revision 23
# baseline (speedup 1.0000x reference)
"""GAT (4-layer, 8-head) + GraphNorm kernel for 8 TRN2 NeuronCores — v2.

Strategy: destination-sharded message passing with ONE collective per layer.
Each core owns N/8 nodes and all edges pointing at them. Per layer, each core
broadcasts bf16(h^T) of its shard PLUS hi/lo-bf16-packed GraphNorm partial
stats in a single AllGather. Every core then (a) reconstructs global stats
and the norm coefficients, (b) locally normalizes + projects ALL 8 gathered
shards into the gather table z = hn @ (Wg @ blockdiag(T_h)) (bf16 rows,
row-major via PE with h as the stationary operand — transpose for free),
(c) normalizes its own shard in f32 (v1 norm_apply). T_h's first column
equals att_src[h] so the per-edge source attention term is z[16h] for free.
Messages are fetched with one dma_gather per 128-node block (negative-int16
index trick to cover >32K rows), attention runs on ACT/DVE, and aggregation
runs on the PE as PSUM accumulation with the msg slot as the stationary
operand (transposed aggregate — saves a per-block transpose). The final
GraphNorm needs a 5th, tiny stats-only AllGather.

Rationale: in this runtime a collective costs ~550us FIXED (launch) +
~6us/MB, so merging the per-layer stats AllReduce (434us) into the table
AllGather and keeping 5 total launches dominates all other wins.
"""

import sys

import numpy as np

if "/opt/trn_rl_repo" not in sys.path:
    sys.path.insert(0, "/opt/trn_rl_repo")

# ---------------------------------------------------------------- config

N_CORES = 8
H = 8
DH = 16
HID = 128
EPS = 1e-5
NEG_SLOPE = 0.2
PAD_ES = -80.0  # es value stored in the pad table row; kills pad-slot alphas

FULL_CFG = dict(N=50000, E=800000, G=16, L=4)

# Optional explicit per-gather SWDGE queue sequence (emission order). Used by
# profile_sim.py to match CoreSim's DMASW-lane/queue locking; None on HW.
QUEUE_SEQ = None


def _derive(cfg):
    N = cfg["N"]
    npc = N // N_CORES                      # real nodes per core
    nblk = (npc + 127) // 128               # 128-node blocks per core
    npad = nblk * 128                       # padded nodes per core
    trows = N_CORES * npad                  # global table rows
    tbase = max(0, trows - 32767)           # gather base row (neg-idx trick)
    assert npc < npad, "need at least one pad node for the pad table row"
    pad_row = trows - 1                     # last pad node of core 7
    assert pad_row - tbase >= 0
    return dict(NPC=npc, NBLK=nblk, NPAD=npad, TROWS=trows, TBASE=tbase,
                PAD_ROW=pad_row, XW=npad + 4 * cfg["G"], **cfg)


# ---------------------------------------------------------- host preprocess


def _padrow():
    import ml_dtypes
    r = np.zeros((1, HID), np.float32)
    r[0, 0::DH] = PAD_ES
    return r.astype(ml_dtypes.bfloat16)


def _preprocess(inputs, cfg):
    """All numpy. Returns per-core data + SPMD-uniform shape info."""
    d = _derive(cfg)
    N, G, L = d["N"], d["G"], d["L"]
    NPC, NBLK, NPAD, TBASE, PAD_ROW = (
        d["NPC"], d["NBLK"], d["NPAD"], d["TBASE"], d["PAD_ROW"])

    x = np.asarray(inputs["x"], np.float32)
    ei = np.asarray(inputs["edge_index"], np.int64)
    batch = np.asarray(inputs["batch"], np.int64).astype(np.int32)
    loops = np.arange(N, dtype=np.int64)
    src = np.concatenate([ei[0], loops]).astype(np.int64)
    dst = np.concatenate([ei[1], loops]).astype(np.int64)

    # ---- per-core node permutation: degree-sort (desc) within graph runs
    deg_all = np.bincount(dst, minlength=N)
    perms = []          # perms[c][new_pos] = orig local id
    for c in range(N_CORES):
        lo, hi = c * NPC, (c + 1) * NPC
        b = batch[lo:hi]
        degl = deg_all[lo:hi]
        order = np.lexsort((-degl, b))
        perms.append(order.astype(np.int64))
    inv_perms = [np.argsort(p) for p in perms]

    row_of = np.empty(N, np.int64)
    for c in range(N_CORES):
        row_of[c * NPC:(c + 1) * NPC] = c * NPAD + inv_perms[c]

    # ---- per-core edge slot grids
    deg_perm = [deg_all[c * NPC:(c + 1) * NPC][perms[c]] for c in range(N_CORES)]
    d_i = np.zeros(NBLK, np.int64)
    for c in range(N_CORES):
        dp = np.zeros(NPAD, np.int64)
        dp[:NPC] = deg_perm[c]
        d_i = np.maximum(d_i, dp.reshape(NBLK, 128).max(1))
    d_i = np.maximum(d_i, 1)

    pad_idx = PAD_ROW - TBASE
    core_edges = []
    for c in range(N_CORES):
        m = (dst >= c * NPC) & (dst < (c + 1) * NPC)
        s_c = src[m]
        dloc = inv_perms[c][dst[m] - c * NPC]
        core_edges.append((dloc, s_c))

    # gather units of up to 8 slot-cols (1024 idxs — the SWDGE ring limit)
    units = []
    for i in range(NBLK):
        s0 = 0
        while s0 < d_i[i]:
            nc_ = min(8, d_i[i] - s0)
            units.append((i, s0, int(nc_)))
            s0 += nc_
    idx_w = sum(u[2] * 8 for u in units)          # int16 cols in wrapped layout

    idx_arrs = []
    for c in range(N_CORES):
        dloc, s_c = core_edges[c]
        grid = np.full((NBLK, 128, int(d_i.max())), pad_idx, np.int64)
        order = np.argsort(dloc, kind="stable")
        dloc_s, src_s = dloc[order], s_c[order]
        slot = np.zeros(len(dloc_s), np.int64)
        if len(dloc_s):
            new_node = np.r_[True, dloc_s[1:] != dloc_s[:-1]]
            idx0 = np.flatnonzero(new_node)
            counts = np.diff(np.r_[idx0, len(dloc_s)])
            slot = np.arange(len(dloc_s)) - np.repeat(idx0, counts)
        blk = dloc_s // 128
        j = dloc_s % 128
        grid[blk, j, slot] = row_of[src_s] - TBASE
        assert grid.min() >= -32768 and grid.max() <= 32767

        # tail-strip safety: last idx of every gather unit must be >= 0.
        for (i, s0, ncl) in units:
            last_col = s0 + ncl - 1
            if grid[i, 127, last_col] < 0:
                row = grid[i, 127, :d_i[i]]
                cand = np.flatnonzero(row >= 0)
                if len(cand):
                    k = cand[0]
                    row[last_col], row[k] = row[k], row[last_col]
                else:
                    raise RuntimeError("cannot fix tail-strip; "
                                       "all-negative node row")
        # wrapped int16 layout per unit: flat i -> [i%16, i//16], 8x replicated
        parts = []
        for (i, s0, ncl) in units:
            flat = grid[i, :, s0:s0 + ncl].T.reshape(-1)   # slot-major
            w = flat.reshape(-1, 16).T                     # [16, n/16]
            parts.append(np.tile(w, (8, 1)))
        idx_arrs.append(np.concatenate(parts, axis=1).astype(np.int16))

    # ---- graph one-hot tiles
    g1h = []
    g1ht = []
    cnt = np.bincount(batch, minlength=G).astype(np.float64)
    for c in range(N_CORES):
        bperm = batch[c * NPC:(c + 1) * NPC][perms[c]]
        gm = np.zeros((NPAD, G), np.float32)
        gm[np.arange(NPC), bperm] = 1.0
        gmb = gm.reshape(NBLK, 128, G)
        g1h.append(np.ascontiguousarray(
            gmb.transpose(1, 0, 2).reshape(128, NBLK * G)))
        g1ht.append(np.ascontiguousarray(gm.T))
    # global one-hot over table layout [G, TROWS] (same for all cores)
    import ml_dtypes
    g1ht8 = np.concatenate(g1ht, axis=1).astype(ml_dtypes.bfloat16)
    assert g1ht8.shape == (G, N_CORES * NPAD)
    # per-shard graph runs over permuted+padded columns (same on all cores)
    segs8 = []
    for c in range(N_CORES):
        bp = batch[c * NPC:(c + 1) * NPC][perms[c]]
        bp_full = np.concatenate([bp, np.full(NPAD - NPC, bp[-1], bp.dtype)])
        bounds = ([0] + list(np.flatnonzero(np.diff(bp_full)) + 1) + [NPAD])
        segs8.append([(int(bounds[k]), int(bounds[k + 1]),
                       int(bp_full[bounds[k]]))
                      for k in range(len(bounds) - 1)])

    # x transposed + permuted + padded
    xT = []
    for c in range(N_CORES):
        xp = np.zeros((NPAD, x.shape[1]), np.float32)
        xp[:NPC] = x[c * NPC:(c + 1) * NPC][perms[c]]
        xT.append(np.ascontiguousarray(xp.T))

    # ---- weights
    in_W = np.asarray(inputs["in_W"], np.float32)
    in_b = np.asarray(inputs["in_b"], np.float32)
    Wg = np.asarray(inputs["Wg"], np.float32)
    att_src = np.asarray(inputs["att_src"], np.float32)
    att_dst = np.asarray(inputs["att_dst"], np.float32)
    gat_b = np.asarray(inputs["gat_b"], np.float32)
    gn_w = np.asarray(inputs["gn_w"], np.float32)
    gn_b = np.asarray(inputs["gn_b"], np.float32)
    gn_s = np.asarray(inputs["gn_s"], np.float32)

    W_z = np.zeros((L, HID, HID), np.float32)
    W_ed = np.zeros((L, HID, H), np.float32)
    Tinv_bd = np.zeros((L, HID, HID), np.float32)
    for l in range(L):
        for h in range(H):
            a = att_src[l, h]
            rng = np.random.default_rng(1234 + l * 16 + h)
            M = np.concatenate([a[:, None],
                                rng.standard_normal((DH, DH - 1))], 1)
            q, _ = np.linalg.qr(M)
            T = np.concatenate([a[:, None], q[:, 1:]], 1)  # [16,16]
            Ti = np.linalg.inv(T)
            sl = slice(h * DH, (h + 1) * DH)
            W_z[l][:, sl] = Wg[l][:, sl] @ T
            Tinv_bd[l][sl, sl] = Ti
            W_ed[l][:, h] = Wg[l][:, sl] @ att_dst[l, h]

    cnt_recip = np.zeros(G, np.float32)
    nz = cnt > 0
    cnt_recip[nz] = (1.0 / cnt[nz]).astype(np.float32)

    s = gn_s
    s2c = 2.0 * s - s * s

    consts = dict(
        inw=in_W,
        inb=in_b.reshape(HID, 1),
        wz=W_z, wed=W_ed, tinv=Tinv_bd,
        gatb=np.ascontiguousarray(gat_b.T),
        gnw=np.ascontiguousarray(gn_w.T),
        gnb=np.ascontiguousarray(gn_b.T),
        gns=np.ascontiguousarray(s.T),
        gns2c=np.ascontiguousarray(s2c.T),
        cntr=np.tile(cnt_recip[None, :], (HID, 1)),
        ident=np.eye(HID, dtype=np.float32),
        padrow=_padrow(),
    )

    return dict(d=d, units=units, d_i=d_i, idx_w=idx_w,
                idx_arrs=idx_arrs, g1h=g1h, g1ht=g1ht, g1ht8=g1ht8,
                segs8=segs8, xT=xT,
                perms=perms, inv_perms=inv_perms, consts=consts,
                batch=batch)


# ------------------------------------------------- numpy device emulation
# Mirrors the v2 device program (layouts, pads, bf16 rounding points) so host
# logic can be validated without a compile.


def _bf16(a):
    import ml_dtypes
    return a.astype(ml_dtypes.bfloat16).astype(np.float32)


def _coefs(tot, C, lidx, G):
    """tot: [HID, 2G] global stats; returns c1, c0 [HID, G] (f32)."""
    mean = tot[:, :G] * C["cntr"]
    ex2 = tot[:, G:] * C["cntr"]
    var = ex2 - C["gns2c"][:, lidx:lidx + 1] * mean * mean
    rstd = 1.0 / np.sqrt(var + EPS)
    c1 = C["gnw"][:, lidx:lidx + 1] * rstd
    c0 = C["gnb"][:, lidx:lidx + 1] - C["gns"][:, lidx:lidx + 1] * mean * c1
    return c1, c0


def _numpy_pipeline(prep, dbg=None):
    d = prep["d"]
    L, G = d["L"], d["G"]
    NPC, NBLK, NPAD, TROWS, TBASE = (
        d["NPC"], d["NBLK"], d["NPAD"], d["TROWS"], d["TBASE"])
    C = prep["consts"]
    units, d_i = prep["units"], prep["d_i"]
    g1ht8 = prep["g1ht8"]

    # input proj (per core, [128 f, NPAD n])
    hT = [C["inw"].T @ prep["xT"][c] + C["inb"] for c in range(N_CORES)]
    if dbg is not None:
        dbg["h0"] = [h.copy() for h in hT]

    stats = None          # [N_CORES, HID, 2G] partials of current pre-norm h
    for l in range(L):
        # ---- exchange: hb8 = bf16(h^T) of all shards; stats hi/lo packed
        hb8 = [_bf16(hT[c]) for c in range(N_CORES)]
        if l >= 1:
            tot = np.zeros((HID, 2 * G), np.float32)
            for c in range(N_CORES):
                hi = _bf16(stats[c])
                lo = _bf16(stats[c] - hi)
                tot += hi + lo
            c1, c0 = _coefs(tot, C, l - 1, G)
            if dbg is not None and l == 1:
                dbg["st1"] = tot.copy()
            table_coefs = (c1, c0)
            c1b, c0b = _bf16(c1), _bf16(c0)
            for c in range(N_CORES):
                p1 = c1 @ prep["g1ht"][c]
                p0 = c0 @ prep["g1ht"][c]
                hT[c] = hT[c] * p1 + p0
        else:
            table_coefs = None

        # ---- table rows
        wzb = _bf16(C["wz"][l])
        tbl = np.zeros((TROWS, HID), np.float32)
        if table_coefs is None:
            for c in range(N_CORES):
                tbl[c * NPAD:(c + 1) * NPAD] = _bf16(hb8[c].T @ wzb)
        else:
            for c in range(N_CORES):
                hn = _bf16(_bf16(hb8[c] * (c1b @ prep["g1ht"][c]))
                           + (c0b @ prep["g1ht"][c]))
                tbl[c * NPAD:(c + 1) * NPAD] = _bf16(hn.T @ wzb)
        tbl[TROWS - 1] = 0.0
        tbl[TROWS - 1, 0::DH] = PAD_ES
        if dbg is not None and l == 0:
            dbg["tbl0"] = tbl.copy()

        # ---- ed from own (f32) normalized h
        eds = [C["wed"][l].T @ hT[c] for c in range(N_CORES)]   # [H, NPAD]
        if dbg is not None and l == 0:
            dbg["ed0"] = [e.copy() for e in eds]

        # ---- edge phase per core
        new_stats = np.zeros((N_CORES, HID, 2 * G), np.float32)
        for c in range(N_CORES):
            idx = prep["idx_arrs"][c]
            col = 0
            for i in range(NBLK):
                di = int(d_i[i])
                w = idx[:16, col:col + di * 8]
                col += di * 8
                flat = w.T.reshape(-1)
                rows = tbl[flat.astype(np.int64) + TBASE]
                msg = rows.reshape(di, 128, HID).transpose(1, 0, 2)
                es = msg[:, :, 0::DH]                      # [128, di, H]
                ed = eds[c][:, i * 128:(i + 1) * 128].T    # [128, H]
                e = es + ed[:, None, :]
                e = np.where(e >= 0, e, NEG_SLOPE * e)
                exb = _bf16(np.exp(e))
                den = exb.sum(1)                           # [128, H]
                alph = _bf16(exb / den[:, None, :])
                mp = _bf16(msg * alph.repeat(DH, axis=2))
                agg = mp.sum(1)                            # [128 j, 128 f]
                attnT = C["tinv"][l].T @ agg.T             # [f', j]
                sl = slice(i * 128, (i + 1) * 128)
                xt = attnT + C["gatb"][:, l:l + 1]
                hT[c][:, sl] = xt + hT[c][:, sl]
                g1hb = prep["g1h"][c][:, i * G:(i + 1) * G]
                xb = hT[c][:, sl].T                        # [j, f]
                new_stats[c, :, :G] += (xb.T @ g1hb)
                new_stats[c, :, G:] += ((xb * xb).T @ g1hb)
            if dbg is not None and l == 0 and c == 0:
                dbg["x0"] = hT[0].copy()
        stats = new_stats

    # ---- final norm
    tot = np.zeros((HID, 2 * G), np.float32)
    for c in range(N_CORES):
        hi = _bf16(stats[c])
        lo = _bf16(stats[c] - hi)
        tot += hi + lo
    c1, c0 = _coefs(tot, C, L - 1, G)
    for c in range(N_CORES):
        p1 = c1 @ prep["g1ht"][c]
        p0 = c0 @ prep["g1ht"][c]
        hT[c] = hT[c] * p1 + p0

    # ---- output assembly
    N = d["N"]
    out = np.zeros((N, HID), np.float32)
    for c in range(N_CORES):
        out[c * NPC + prep["perms"][c]] = hT[c][:, :NPC].T
    return out


# ---------------------------------------------------------------- device


def _build_program(prep, timing_reps=None, dbg=True, ablate=()):
    import contextlib

    import concourse.tile as tile
    from concourse import bacc, mybir, library_config

    d = prep["d"]
    L, G = d["L"], d["G"]
    NPC, NBLK, NPAD, TROWS, TBASE, XW = (
        d["NPC"], d["NBLK"], d["NPAD"], d["TROWS"], d["TBASE"], d["XW"])
    units, d_i, idx_w = prep["units"], prep["d_i"], prep["idx_w"]
    DIMAX = int(d_i.max())
    F_IN = prep["xT"][0].shape[0]
    NCHUNK = (NPAD + 511) // 512
    AF = mybir.ActivationFunctionType
    ALU = mybir.AluOpType

    f32, bf16, i16 = mybir.dt.float32, mybir.dt.bfloat16, mybir.dt.int16

    nc = bacc.Bacc(None, target_bir_lowering=False, num_swdge_queues=4)

    def param(name, shape, dtype=f32, out=False):
        return nc.declare_dram_parameter(name, list(shape), dtype, isOutput=out)

    P = dict(
        xT=param("xT", [F_IN, NPAD]),
        idx=param("idx", [128, idx_w], i16),
        g1h=param("g1h", [128, NBLK * G]),
        g1ht=param("g1ht", [G, NBLK * 128]),
        g1ht8=param("g1ht8", [G, TROWS], bf16),
        inw=param("inw", [F_IN, HID]),
        inb=param("inb", [HID, 1]),
        wz=param("wz", [L, HID, HID]),
        wed=param("wed", [L, HID, H]),
        tinv=param("tinv", [L, HID, HID]),
        gatb=param("gatb", [HID, L]),
        gnw=param("gnw", [HID, L]),
        gnb=param("gnb", [HID, L]),
        gns=param("gns", [HID, L]),
        gns2c=param("gns2c", [HID, L]),
        cntr=param("cntr", [HID, G]),
        ident=param("ident", [HID, HID]),
        padrow=param("padrow", [1, HID], bf16),
        out=param("out", [NPC, HID], out=True),
        dbg_h0=param("dbg_h0", [HID, NPAD], out=True),
        dbg_x0=param("dbg_x0", [HID, NPAD], out=True),
        dbg_tbl=param("dbg_tbl", [TROWS, HID], bf16, out=True),
        dbg_ed=param("dbg_ed", [128, NBLK * H], out=True),
        dbg_st=param("dbg_st", [HID, 2 * G], out=True),
    )

    qctr = [0]

    def next_q():
        if QUEUE_SEQ is not None:
            q = QUEUE_SEQ[qctr[0] % len(QUEUE_SEQ)]
        else:
            q = qctr[0] % 4
        qctr[0] += 1
        return q

    with tile.TileContext(nc) as tc:
        est = contextlib.ExitStack()
        singles = est.enter_context(tc.tile_pool(name="singles", bufs=1))
        msgpA = est.enter_context(tc.tile_pool(name="msgA", bufs=4))
        msgpB = est.enter_context(tc.tile_pool(name="msgB", bufs=8))
        ep = est.enter_context(tc.tile_pool(name="etile", bufs=5))
        blkp = est.enter_context(tc.tile_pool(name="blk", bufs=5))
        stag = est.enter_context(tc.tile_pool(name="stag", bufs=2))
        cons = est.enter_context(tc.tile_pool(name="cons", bufs=2))
        psA = est.enter_context(tc.tile_pool(name="psA", bufs=3, space="PSUM"))
        psnp = est.enter_context(tc.tile_pool(name="psnp", bufs=2, space="PSUM"))
        pssm = est.enter_context(tc.tile_pool(name="pssm", bufs=2, space="PSUM"))
        psacc = est.enter_context(tc.tile_pool(name="psacc", bufs=1, space="PSUM"))
        dram = est.enter_context(tc.tile_pool(name="dram", bufs=1, space="DRAM"))

        nc.gpsimd.load_library(library_config.mlp)

        # ---------------- constants
        def load(t, src):
            nc.sync.dma_start(out=t, in_=src)
            return t

        ident_f = load(singles.tile([HID, HID], f32, name="idf"), P["ident"][:])
        ident_b = singles.tile([HID, HID], bf16, name="idb")
        nc.vector.tensor_copy(out=ident_b, in_=ident_f)
        idx_sb = load(singles.tile([128, idx_w], i16, name="idxs"), P["idx"][:])
        g1h_sb = load(singles.tile([128, NBLK * G], f32, name="g1h"), P["g1h"][:])
        inw_sb = load(singles.tile([F_IN, HID], f32, name="inw"), P["inw"][:])
        inb_sb = load(singles.tile([HID, 1], f32, name="inb"), P["inb"][:])
        wz_sb = [load(singles.tile([HID, HID], f32, name=f"wz{l}"), P["wz"][l])
                 for l in range(L)]
        wzb_sb = []
        for l in range(L):
            t = singles.tile([HID, HID], bf16, name=f"wzb{l}")
            nc.vector.tensor_copy(out=t, in_=wz_sb[l])
            wzb_sb.append(t)
        wed_sb = [load(singles.tile([HID, H], f32, name=f"wed{l}"), P["wed"][l])
                  for l in range(L)]
        tinv_sb = [load(singles.tile([HID, HID], f32, name=f"ti{l}"), P["tinv"][l])
                   for l in range(L)]
        gatb_sb = load(singles.tile([HID, L], f32, name="gatb"), P["gatb"][:])
        gnw_sb = load(singles.tile([HID, L], f32, name="gnw"), P["gnw"][:])
        gnb_sb = load(singles.tile([HID, L], f32, name="gnb"), P["gnb"][:])
        gns_sb = load(singles.tile([HID, L], f32, name="gns"), P["gns"][:])
        gns2c_sb = load(singles.tile([HID, L], f32, name="gns2c"), P["gns2c"][:])
        cntr_sb = load(singles.tile([HID, G], f32, name="cntr"), P["cntr"][:])
        eps_sb = singles.tile([HID, 1], f32, name="eps")
        nc.vector.memset(eps_sb, EPS)
        ones1 = singles.tile([1, HID], bf16, name="ones1")
        nc.vector.memset(ones1, 1.0)

        h_a = singles.tile([HID, NPAD], f32, name="h_a")
        ed_all = singles.tile([128, NBLK * H], f32, name="ed_all")

        hx_in = [dram.tile([128, XW], bf16, name=f"hxi{l}") for l in range(L)]
        hx_out = [dram.tile([N_CORES * 128, XW], bf16, addr_space="Shared",
                            name=f"hxo{l}") for l in range(L)]
        sx_in = dram.tile([128, 4 * G], bf16, name="sxi")
        sx_out = dram.tile([N_CORES * 128, 4 * G], bf16, addr_space="Shared",
                           name="sxo")
        tblr = [dram.tile([TROWS, HID], bf16, name=f"tbl{l}") for l in range(L)]

        # ---------------- helpers

        def inproj():
            for k in range(NCHUNK):
                c0, c1_ = k * 512, min((k + 1) * 512, NPAD)
                w = c1_ - c0
                xt = stag.tile([F_IN, 512], f32, name="xchunk")
                nc.sync.dma_start(out=xt[:, :w], in_=P["xT"][:, c0:c1_])
                ps = psA.tile([HID, 512], f32, name="psbig")
                nc.tensor.matmul(out=ps[:, :w], lhsT=inw_sb, rhs=xt[:, :w],
                                 start=True, stop=True)
                nc.scalar.activation(out=h_a[:, c0:c1_], in_=ps[:, :w],
                                     func=AF.Identity, bias=inb_sb[:, 0:1])

        def producer(l):
            """bf16 copies of h_a -> hx_in[l][:, :NPAD]."""
            for k in range(NCHUNK):
                c0, c1_ = k * 512, min((k + 1) * 512, NPAD)
                w = c1_ - c0
                hxb = stag.tile([128, 512], bf16, name="hxb")
                nc.scalar.activation(out=hxb[:, :w], in_=h_a[:, c0:c1_],
                                     func=AF.Copy)
                nc.sync.dma_start(out=hx_in[l][:, c0:c1_], in_=hxb[:, :w])

        def exchange(l):
            if timing_reps is None:
                nc.gpsimd.collective_compute(
                    "AllGather", mybir.AluOpType.bypass,
                    replica_groups=[list(range(N_CORES))],
                    ins=[hx_in[l].opt()], outs=[hx_out[l].opt()])
            else:
                nc.sync.dma_start(out=hx_out[l][:128, :], in_=hx_in[l][:])

        def exchange_final():
            if timing_reps is None:
                nc.gpsimd.collective_compute(
                    "AllGather", mybir.AluOpType.bypass,
                    replica_groups=[list(range(N_CORES))],
                    ins=[sx_in.opt()], outs=[sx_out.opt()])
            else:
                nc.sync.dma_start(out=sx_out[:128, :], in_=sx_in[:])

        def stats_coefs(stats_src, lidx):
            """Reconstruct global stats from 8 hi/lo shards; return c1T, c0T
            ([G, HID] f32 SBUF) and the [HID, G] c1/c0 tiles."""
            sacc = blkp.tile([128, 2 * G], f32, name="sacc")
            shl = cons.tile([128, N_CORES, 4 * G], bf16, name="shl")
            nc.sync.dma_start(
                out=shl, in_=stats_src().rearrange("(c p) w -> p c w", p=128))
            s64 = blkp.tile([128, 4 * G], f32, name="s64")
            nc.vector.tensor_reduce(
                out=s64, in_=shl.rearrange("p c w -> p w c"),
                axis=mybir.AxisListType.X, op=ALU.add)
            nc.vector.tensor_tensor(out=sacc, in0=s64[:, :2 * G],
                                    in1=s64[:, 2 * G:], op=ALU.add)
            if dbg and lidx == 0:
                nc.sync.dma_start(out=P["dbg_st"][:], in_=sacc)
            mean = blkp.tile([128, G], f32, name="mean")
            nc.vector.tensor_tensor(out=mean, in0=sacc[:, :G], in1=cntr_sb,
                                    op=ALU.mult)
            ex2 = blkp.tile([128, G], f32, name="ex2")
            nc.vector.tensor_tensor(out=ex2, in0=sacc[:, G:], in1=cntr_sb,
                                    op=ALU.mult)
            m2 = blkp.tile([128, G], f32, name="m2")
            nc.vector.tensor_tensor(out=m2, in0=mean, in1=mean, op=ALU.mult)
            nc.vector.tensor_tensor(
                out=m2, in0=m2,
                in1=gns2c_sb[:, lidx:lidx + 1].to_broadcast([HID, G]),
                op=ALU.mult)
            var = blkp.tile([128, G], f32, name="var")
            nc.vector.tensor_tensor(out=var, in0=ex2, in1=m2, op=ALU.subtract)
            nc.scalar.activation(out=var, in_=var, func=AF.Sqrt,
                                 bias=eps_sb[:, 0:1])
            rstd = blkp.tile([128, G], f32, name="rstd")
            nc.vector.reciprocal(out=rstd, in_=var)
            c1 = blkp.tile([128, G], f32, name="c1")
            nc.vector.tensor_tensor(
                out=c1, in0=rstd,
                in1=gnw_sb[:, lidx:lidx + 1].to_broadcast([HID, G]),
                op=ALU.mult)
            c0t = blkp.tile([128, G], f32, name="c0t")
            nc.vector.tensor_tensor(out=c0t, in0=mean, in1=c1, op=ALU.mult)
            nc.vector.tensor_tensor(
                out=c0t, in0=c0t,
                in1=gns_sb[:, lidx:lidx + 1].to_broadcast([HID, G]),
                op=ALU.mult)
            c0 = blkp.tile([128, G], f32, name="c0")
            nc.vector.tensor_tensor(
                out=c0, in0=gnb_sb[:, lidx:lidx + 1].to_broadcast([HID, G]),
                in1=c0t, op=ALU.subtract)
            pc = pssm.tile([128, HID], f32, name="pssm")
            nc.tensor.matmul(out=pc[:G, :], lhsT=c1, rhs=ident_f,
                             start=True, stop=True)
            c1T = blkp.tile([G, HID], f32, name="c1T")
            nc.vector.tensor_copy(out=c1T, in_=pc[:G, :])
            c1Tb = blkp.tile([G, HID], bf16, name="c1Tb")
            nc.vector.tensor_copy(out=c1Tb, in_=pc[:G, :])
            pc2 = pssm.tile([128, HID], f32, name="pssm")
            nc.tensor.matmul(out=pc2[:G, :], lhsT=c0, rhs=ident_f,
                             start=True, stop=True)
            c0T = blkp.tile([G, HID], f32, name="c0T")
            nc.vector.tensor_copy(out=c0T, in_=pc2[:G, :])
            c0Tb = blkp.tile([G, HID], bf16, name="c0Tb")
            nc.vector.tensor_copy(out=c0Tb, in_=pc2[:G, :])
            return dict(c1T=c1T, c0T=c0T, c1Tb=c1Tb, c0Tb=c0Tb)

        CW = 2048

        def table_pass(l, coefs):
            """Build tblr[l] from hx_out[l] (all 8 shards). DMAs move
            CW-column chunks; compute runs in 512-col slices."""
            for c in range(N_CORES):
                r0 = c * 128
                for kk in range((NPAD + CW - 1) // CW):
                    b0, b1 = kk * CW, min((kk + 1) * CW, NPAD)
                    wa = b1 - b0
                    hb = cons.tile([128, CW], bf16, name="hb")
                    nc.sync.dma_start(out=hb[:, :wa],
                                      in_=hx_out[l][r0:r0 + 128, b0:b1])
                    if coefs is not None:
                        g1 = cons.tile([G, CW], bf16, name="g1c")
                        nc.sync.dma_start(
                            out=g1[:, :wa],
                            in_=P["g1ht8"][:, c * NPAD + b0:c * NPAD + b1])
                    rows = cons.tile([128, CW], bf16, name="rows")
                    for k2 in range((wa + 511) // 512):
                        o = k2 * 512
                        w = min(512, wa - o)
                        nb = w // 128
                        if coefs is not None:
                            pp1 = psA.tile([128, 512], f32, name="psbig")
                            nc.tensor.matmul(out=pp1[:, :w],
                                             lhsT=coefs["c1Tb"],
                                             rhs=g1[:, o:o + w],
                                             start=True, stop=True)
                            pp0 = psA.tile([128, 512], f32, name="psbig")
                            nc.tensor.matmul(out=pp0[:, :w],
                                             lhsT=coefs["c0Tb"],
                                             rhs=g1[:, o:o + w],
                                             start=True, stop=True)
                            hn = cons.tile([128, 512], bf16, name="hn")
                            nc.vector.tensor_tensor(
                                out=hn[:, :w], in0=hb[:, o:o + w],
                                in1=pp1[:, :w], op=ALU.mult)
                            nc.vector.tensor_tensor(
                                out=hn[:, :w], in0=hn[:, :w],
                                in1=pp0[:, :w], op=ALU.add)
                            hns = hn[:, :w]
                        else:
                            hns = hb[:, o:o + w]
                        zp = psA.tile([128, 512], f32, name="psbig")
                        for j in range(nb):
                            nc.tensor.matmul(
                                out=zp[:, j * 128:(j + 1) * 128],
                                lhsT=hns[:, j * 128:(j + 1) * 128],
                                rhs=wzb_sb[l], start=True, stop=True)
                        nc.scalar.activation(out=rows[:, o:o + w],
                                             in_=zp[:, :w], func=AF.Copy)
                    rr = c * NPAD + b0
                    nc.sync.dma_start(
                        out=tblr[l][rr:rr + wa, :].rearrange(
                            "(b p) f -> p b f", p=128),
                        in_=rows[:, :wa].rearrange("p (b f) -> p b f", f=HID))
            nc.sync.dma_start(out=tblr[l][TROWS - 1:TROWS, :],
                              in_=P["padrow"][:])

        def norm_own(lidx, coefs):
            """f32 GraphNorm of h_a in place (own shard), 512-col chunks."""
            c1T, c0T = coefs["c1T"], coefs["c0T"]
            for k in range(NCHUNK):
                c0_, c1_ = k * 512, min((k + 1) * 512, NPAD)
                w = c1_ - c0_
                g1htb = stag.tile([G, 512], f32, name="g1htb")
                nc.sync.dma_start(out=g1htb[:, :w],
                                  in_=P["g1ht"][:, c0_:c1_])
                p1 = psA.tile([128, 512], f32, name="psbig")
                nc.tensor.matmul(out=p1[:, :w], lhsT=c1T, rhs=g1htb[:, :w],
                                 start=True, stop=True)
                p0 = psA.tile([128, 512], f32, name="psbig")
                nc.tensor.matmul(out=p0[:, :w], lhsT=c0T, rhs=g1htb[:, :w],
                                 start=True, stop=True)
                sl = slice(c0_, c1_)
                nc.vector.tensor_tensor(out=h_a[:, sl], in0=h_a[:, sl],
                                        in1=p1[:, :w], op=ALU.mult)
                nc.vector.tensor_tensor(out=h_a[:, sl], in0=h_a[:, sl],
                                        in1=p0[:, :w], op=ALU.add)

        def ed_pass(l):
            # ed_all[j, i*H+h] = sum_f h_a[f, i*128+j] * wed[f, h]
            for k in range(NCHUNK):
                c0_, c1_ = k * 512, min((k + 1) * 512, NPAD)
                nbb = (c1_ - c0_) // 128
                psd = pssm.tile([128, HID], f32, name="pssm")
                for bb in range(nbb):
                    i = (c0_ + bb * 128) // 128
                    nc.tensor.matmul(
                        out=psd[:, bb * H:(bb + 1) * H],
                        lhsT=h_a[:, i * 128:(i + 1) * 128],
                        rhs=wed_sb[l], start=True, stop=True)
                nc.vector.tensor_copy(
                    out=ed_all[:, (c0_ // 128) * H:(c1_ // 128) * H],
                    in_=psd[:, :nbb * H])

        # ---------------- edge phase (3-stage software pipeline over blocks)
        stats12 = psacc.tile([HID, 2 * G], f32, name="sab")
        stats1 = stats12[:, :G]
        stats2 = stats12[:, G:]

        def edge_phase(l):
            ucol = {}
            col = 0
            for (i, s0, ncl) in units:
                ucol[(i, s0)] = col
                col += ncl * 8
            DMID = min(20, DIMAX)
            state = {}

            def stage1(i):  # gather + attention scalars
                di = int(d_i[i])
                if di > DMID:
                    msg = msgpA.tile([128, DIMAX, HID], bf16, name="msgA")
                else:
                    msg = msgpB.tile([128, DMID, HID], bf16, name="msgB")
                s0 = 0
                while s0 < di:
                    ncl = min(8, di - s0)
                    c0 = ucol[(i, s0)]
                    nidx = ncl * 128
                    if "gather" not in ablate:
                        nc.gpsimd.dma_gather(
                            out_ap=msg[:, s0:s0 + ncl, :],
                            in_ap=tblr[l][TBASE:, :],
                            idxs_ap=idx_sb[:, c0:c0 + ncl * 8],
                            num_idxs=nidx, num_idxs_reg=nidx, elem_size=HID,
                            queue_num=next_q())
                    else:
                        nc.vector.memset(msg[:, s0:s0 + ncl, :], 0.25)
                    s0 += ncl
                e_t = ep.tile([128, DIMAX, H], f32, name="e_t")
                nc.vector.tensor_tensor(
                    out=e_t[:, :di, :],
                    in0=msg[:, :di, 0:HID:DH],
                    in1=ed_all[:, i * H:(i + 1) * H].unsqueeze(1)
                        .to_broadcast([128, di, H]),
                    op=ALU.add)
                # leaky_relu(x) = (1+a)/2*x + (1-a)/2*|x|
                ab_t = ep.tile([128, DIMAX, H], f32, name="ab_t")
                nc.scalar.activation(out=ab_t[:, :di, :], in_=e_t[:, :di, :],
                                     func=AF.Abs,
                                     scale=(1.0 - NEG_SLOPE) / 2.0)
                nc.vector.tensor_scalar_mul(
                    e_t[:, :di, :], e_t[:, :di, :], (1.0 + NEG_SLOPE) / 2.0)
                nc.vector.tensor_tensor(out=e_t[:, :di, :], in0=e_t[:, :di, :],
                                        in1=ab_t[:, :di, :], op=ALU.add)
                exb = ep.tile([128, DIMAX, H], bf16, name="exb")
                nc.scalar.activation(out=exb[:, :di, :], in_=e_t[:, :di, :],
                                     func=AF.Exp)
                den = blkp.tile([128, H], f32, name="den")
                nc.vector.tensor_reduce(
                    out=den, in_=exb[:, :di, :].rearrange("p a b -> p b a"),
                    axis=mybir.AxisListType.X, op=ALU.add)
                rec = blkp.tile([128, H], f32, name="rec")
                nc.vector.reciprocal(out=rec, in_=den)
                alph = ep.tile([128, DIMAX, H], bf16, name="alph")
                nc.vector.tensor_tensor(
                    out=alph[:, :di, :], in0=exb[:, :di, :],
                    in1=rec.unsqueeze(1).to_broadcast([128, di, H]),
                    op=ALU.mult)
                state[i] = (msg, alph)

            def stage2(i):  # weight + transposed aggregate
                di = int(d_i[i])
                msg, alph = state[i]
                nc.vector.tensor_tensor(
                    out=msg[:, :di, :].rearrange("p a (b c) -> p a b c", b=H),
                    in0=msg[:, :di, :].rearrange("p a (b c) -> p a b c", b=H),
                    in1=alph[:, :di, :].unsqueeze(3)
                        .to_broadcast([128, di, H, DH]),
                    op=ALU.mult)
                nps = psnp.tile([128, HID], f32, name="psnp")
                for sj in range(di):
                    nc.tensor.matmul(out=nps, lhsT=msg[:, sj, :], rhs=ident_b,
                                     start=(sj == 0), stop=(sj == di - 1))
                state[i] = nps  # [f, j] transposed aggregate

            def stage3(i):  # unrotate + residual + stats
                nps = state.pop(i)
                aggs = blkp.tile([128, HID], f32, name="aggs")
                nc.scalar.activation(out=aggs, in_=nps, func=AF.Copy)
                pat = pssm.tile([128, HID], f32, name="pssm")
                nc.tensor.matmul(out=pat, lhsT=tinv_sb[l], rhs=aggs,
                                 start=True, stop=True)
                xt = blkp.tile([128, HID], f32, name="xt")
                nc.scalar.activation(out=xt, in_=pat, func=AF.Identity,
                                     bias=gatb_sb[:, l:l + 1])
                sl = slice(i * 128, (i + 1) * 128)
                nc.vector.tensor_tensor(out=h_a[:, sl], in0=xt,
                                        in1=h_a[:, sl], op=ALU.add)
                if "stats" in ablate:
                    return
                pxb = pssm.tile([128, HID], f32, name="pssm")
                nc.tensor.matmul(out=pxb, lhsT=h_a[:, sl], rhs=ident_f,
                                 start=True, stop=True)
                xb = blkp.tile([128, HID], f32, name="xb")
                nc.vector.tensor_copy(out=xb, in_=pxb)
                sq = blkp.tile([128, HID], f32, name="sq")
                nc.scalar.activation(out=sq, in_=xb, func=AF.Square)
                nc.tensor.matmul(out=stats1, lhsT=xb,
                                 rhs=g1h_sb[:, i * G:(i + 1) * G],
                                 start=(i == 0), stop=(i == NBLK - 1),
                                 skip_group_check=True)
                nc.tensor.matmul(out=stats2, lhsT=sq,
                                 rhs=g1h_sb[:, i * G:(i + 1) * G],
                                 start=(i == 0), stop=(i == NBLK - 1),
                                 skip_group_check=True)

            for i in range(NBLK + 4):
                if i < NBLK:
                    stage1(i)
                if 2 <= i <= NBLK + 1 and "post" not in ablate:
                    stage2(i - 2)
                if 4 <= i <= NBLK + 3 and "post" not in ablate:
                    stage3(i - 4)
                if "post" in ablate and i < NBLK:
                    msg, alph = state.pop(i)
                    nc.vector.tensor_copy(
                        out=h_a[:, i * 128 + 0:i * 128 + 8],
                        in_=alph[:, 0, :])

            # pack stats as hi/lo bf16 into the next exchange payload
            stl = stag.tile([HID, 2 * G], f32, name="stl")
            if "post" in ablate or "stats" in ablate:
                nc.vector.memset(stl, 1.0)
            else:
                nc.vector.tensor_copy(out=stl, in_=stats12)
            hilo = stag.tile([HID, 4 * G], bf16, name="hilo")
            nc.scalar.activation(out=hilo[:, :2 * G], in_=stl, func=AF.Copy)
            nc.vector.tensor_tensor(out=hilo[:, 2 * G:], in0=stl,
                                    in1=hilo[:, :2 * G], op=ALU.subtract)
            dst_ap = (hx_in[l + 1][:, NPAD:NPAD + 4 * G] if l < L - 1
                      else sx_in[:, :])
            nc.sync.dma_start(out=dst_ap, in_=hilo)

        def output_pass():
            for k in range(NCHUNK):
                c0_, c1_ = k * 512, min((k + 1) * 512, NPAD)
                r1 = min(c1_, NPC)
                if r1 <= c0_:
                    break
                w = c1_ - c0_
                nbb = w // 128
                po = psA.tile([128, 512], f32, name="psbig")
                for bb in range(nbb):
                    nc.tensor.matmul(
                        out=po[:, bb * 128:(bb + 1) * 128],
                        lhsT=h_a[:, c0_ + bb * 128:c0_ + (bb + 1) * 128],
                        rhs=ident_f, start=True, stop=True)
                rows = stag.tile([128, 512], f32, name="orow")
                nc.vector.tensor_copy(out=rows[:, :w], in_=po[:, :w])
                if r1 == c1_:
                    nc.sync.dma_start(
                        out=P["out"][c0_:c1_, :].rearrange(
                            "(b p) f -> p b f", p=128),
                        in_=rows[:, :w].rearrange("p (b f) -> p b f", f=HID))
                else:
                    # tail: whole 128-blocks then the partial block
                    full = ((r1 - c0_) // 128) * 128
                    if full:
                        nc.sync.dma_start(
                            out=P["out"][c0_:c0_ + full, :].rearrange(
                                "(b p) f -> p b f", p=128),
                            in_=rows[:, :full].rearrange(
                                "p (b f) -> p b f", f=HID))
                    part = r1 - c0_ - full
                    if part:
                        nc.sync.dma_start(
                            out=P["out"][c0_ + full:r1, :],
                            in_=rows[:part, full:full + 128])

        # ---------------- program
        loop_ctx = tc.For_i(0, timing_reps) if timing_reps else None
        if loop_ctx:
            loop_ctx.__enter__()
        inproj()
        if dbg:
            nc.sync.dma_start(out=P["dbg_h0"][:], in_=h_a)
        for l in range(L):
            producer(l)
            exchange(l)
            if l >= 1:
                coefs = stats_coefs(
                    lambda _l=l: hx_out[_l][:, NPAD:NPAD + 4 * G], l - 1)
            else:
                coefs = None
            table_pass(l, coefs)
            if coefs is not None:
                norm_own(l - 1, coefs)
            ed_pass(l)
            if l == 0 and dbg:
                nc.sync.dma_start(out=P["dbg_tbl"][:], in_=tblr[0][:])
                nc.sync.dma_start(out=P["dbg_ed"][:], in_=ed_all)
            edge_phase(l)
            if l == 0 and dbg:
                nc.sync.dma_start(out=P["dbg_x0"][:], in_=h_a)
        exchange_final()
        fcoefs = stats_coefs(lambda: sx_out[:, :], L - 1)
        norm_own(L - 1, fcoefs)
        output_pass()
        if loop_ctx:
            loop_ctx.__exit__(None, None, None)

        est.close()

    nc.compile()
    return nc


def _make_inmaps(prep):
    C = prep["consts"]
    maps = []
    for c in range(N_CORES):
        m = dict(
            xT=prep["xT"][c],
            idx=prep["idx_arrs"][c],
            g1h=prep["g1h"][c],
            g1ht=prep["g1ht"][c],
            g1ht8=prep["g1ht8"],
            inw=C["inw"], inb=C["inb"], wz=C["wz"], wed=C["wed"],
            tinv=C["tinv"], gatb=C["gatb"], gnw=C["gnw"], gnb=C["gnb"],
            gns=C["gns"], gns2c=C["gns2c"], cntr=C["cntr"], ident=C["ident"],
            padrow=C["padrow"],
        )
        maps.append(m)
    return maps


def _assemble(prep, results):
    d = prep["d"]
    NPC = d["NPC"]
    out = np.zeros((d["N"], HID), np.float32)
    for c in range(N_CORES):
        out[c * NPC + prep["perms"][c]] = results[c]["out"]
    return out


def _run(inputs, cfg):
    from concourse.bass_utils import run_bass_kernel_spmd
    prep = _preprocess(inputs, cfg)
    nc = _build_program(prep)
    res = run_bass_kernel_spmd(nc, _make_inmaps(prep),
                               core_ids=list(range(N_CORES)))
    return _assemble(prep, res.results)


def kernel(**inputs):
    return _run(inputs, FULL_CFG)


# revision 25
# speedup vs baseline: 1.0616x; 1.0616x over previous
"""GAT (4-layer, 8-head) + GraphNorm kernel for 8 TRN2 NeuronCores — v2.

Strategy: destination-sharded message passing with ONE collective per layer.
Each core owns N/8 nodes and all edges pointing at them. Per layer, each core
broadcasts bf16(h^T) of its shard PLUS hi/lo-bf16-packed GraphNorm partial
stats in a single AllGather. Every core then (a) reconstructs global stats
and the norm coefficients, (b) locally normalizes + projects ALL 8 gathered
shards into the gather table z = hn @ (Wg @ blockdiag(T_h)) (bf16 rows,
row-major via PE with h as the stationary operand — transpose for free),
(c) normalizes its own shard in f32 (v1 norm_apply). T_h's first column
equals att_src[h] so the per-edge source attention term is z[16h] for free.
Messages are fetched with one dma_gather per 128-node block (negative-int16
index trick to cover >32K rows), attention runs on ACT/DVE, and aggregation
runs on the PE as PSUM accumulation with the msg slot as the stationary
operand (transposed aggregate — saves a per-block transpose). The final
GraphNorm needs a 5th, tiny stats-only AllGather.

Rationale: in this runtime a collective costs ~550us FIXED (launch) +
~6us/MB, so merging the per-layer stats AllReduce (434us) into the table
AllGather and keeping 5 total launches dominates all other wins.
"""

import sys

import numpy as np

if "/opt/trn_rl_repo" not in sys.path:
    sys.path.insert(0, "/opt/trn_rl_repo")

# ---------------------------------------------------------------- config

N_CORES = 8
H = 8
DH = 16
HID = 128
EPS = 1e-5
NEG_SLOPE = 0.2
PAD_ES = -80.0  # es value stored in the pad table row; kills pad-slot alphas

FULL_CFG = dict(N=50000, E=800000, G=16, L=4)

# Optional explicit per-gather SWDGE queue sequence (emission order). Used by
# profile_sim.py to match CoreSim's DMASW-lane/queue locking; None on HW.
QUEUE_SEQ = None


def _derive(cfg):
    N = cfg["N"]
    npc = N // N_CORES                      # real nodes per core
    nblk = (npc + 127) // 128               # 128-node blocks per core
    npad = nblk * 128                       # padded nodes per core
    trows = N_CORES * npad                  # global table rows
    tbase = max(0, trows - 32767)           # gather base row (neg-idx trick)
    assert npc < npad, "need at least one pad node for the pad table row"
    pad_row = trows - 1                     # last pad node of core 7
    assert pad_row - tbase >= 0
    return dict(NPC=npc, NBLK=nblk, NPAD=npad, TROWS=trows, TBASE=tbase,
                PAD_ROW=pad_row, XW=npad + 4 * cfg["G"], **cfg)


# ---------------------------------------------------------- host preprocess


def _padrow():
    import ml_dtypes
    r = np.zeros((1, HID), np.float32)
    r[0, 0::DH] = PAD_ES
    return r.astype(ml_dtypes.bfloat16)


def _preprocess(inputs, cfg):
    """All numpy. Returns per-core data + SPMD-uniform shape info."""
    d = _derive(cfg)
    N, G, L = d["N"], d["G"], d["L"]
    NPC, NBLK, NPAD, TBASE, PAD_ROW = (
        d["NPC"], d["NBLK"], d["NPAD"], d["TBASE"], d["PAD_ROW"])

    x = np.asarray(inputs["x"], np.float32)
    ei = np.asarray(inputs["edge_index"], np.int64)
    batch = np.asarray(inputs["batch"], np.int64).astype(np.int32)
    loops = np.arange(N, dtype=np.int64)
    src = np.concatenate([ei[0], loops]).astype(np.int64)
    dst = np.concatenate([ei[1], loops]).astype(np.int64)

    # ---- per-core node permutation: degree-sort (desc) within graph runs
    deg_all = np.bincount(dst, minlength=N)
    perms = []          # perms[c][new_pos] = orig local id
    for c in range(N_CORES):
        lo, hi = c * NPC, (c + 1) * NPC
        b = batch[lo:hi]
        degl = deg_all[lo:hi]
        order = np.lexsort((-degl, b))
        perms.append(order.astype(np.int64))
    inv_perms = [np.argsort(p) for p in perms]

    row_of = np.empty(N, np.int64)
    for c in range(N_CORES):
        row_of[c * NPC:(c + 1) * NPC] = c * NPAD + inv_perms[c]

    # ---- per-core edge slot grids
    deg_perm = [deg_all[c * NPC:(c + 1) * NPC][perms[c]] for c in range(N_CORES)]
    d_i = np.zeros(NBLK, np.int64)
    for c in range(N_CORES):
        dp = np.zeros(NPAD, np.int64)
        dp[:NPC] = deg_perm[c]
        d_i = np.maximum(d_i, dp.reshape(NBLK, 128).max(1))
    d_i = np.maximum(d_i, 1)

    pad_idx = PAD_ROW - TBASE
    core_edges = []
    for c in range(N_CORES):
        m = (dst >= c * NPC) & (dst < (c + 1) * NPC)
        s_c = src[m]
        dloc = inv_perms[c][dst[m] - c * NPC]
        core_edges.append((dloc, s_c))

    # gather units of up to 8 slot-cols (1024 idxs — the SWDGE ring limit)
    units = []
    for i in range(NBLK):
        s0 = 0
        while s0 < d_i[i]:
            nc_ = min(8, d_i[i] - s0)
            units.append((i, s0, int(nc_)))
            s0 += nc_
    idx_w = sum(u[2] * 8 for u in units)          # int16 cols in wrapped layout

    idx_arrs = []
    for c in range(N_CORES):
        dloc, s_c = core_edges[c]
        grid = np.full((NBLK, 128, int(d_i.max())), pad_idx, np.int64)
        order = np.argsort(dloc, kind="stable")
        dloc_s, src_s = dloc[order], s_c[order]
        slot = np.zeros(len(dloc_s), np.int64)
        if len(dloc_s):
            new_node = np.r_[True, dloc_s[1:] != dloc_s[:-1]]
            idx0 = np.flatnonzero(new_node)
            counts = np.diff(np.r_[idx0, len(dloc_s)])
            slot = np.arange(len(dloc_s)) - np.repeat(idx0, counts)
        blk = dloc_s // 128
        j = dloc_s % 128
        grid[blk, j, slot] = row_of[src_s] - TBASE
        assert grid.min() >= -32768 and grid.max() <= 32767

        # tail-strip safety: last idx of every gather unit must be >= 0.
        for (i, s0, ncl) in units:
            last_col = s0 + ncl - 1
            if grid[i, 127, last_col] < 0:
                row = grid[i, 127, :d_i[i]]
                cand = np.flatnonzero(row >= 0)
                if len(cand):
                    k = cand[0]
                    row[last_col], row[k] = row[k], row[last_col]
                else:
                    raise RuntimeError("cannot fix tail-strip; "
                                       "all-negative node row")
        # wrapped int16 layout per unit: flat i -> [i%16, i//16], 8x replicated
        parts = []
        for (i, s0, ncl) in units:
            flat = grid[i, :, s0:s0 + ncl].T.reshape(-1)   # slot-major
            w = flat.reshape(-1, 16).T                     # [16, n/16]
            parts.append(np.tile(w, (8, 1)))
        idx_arrs.append(np.concatenate(parts, axis=1).astype(np.int16))

    # ---- graph one-hot tiles
    g1h = []
    g1ht = []
    cnt = np.bincount(batch, minlength=G).astype(np.float64)
    for c in range(N_CORES):
        bperm = batch[c * NPC:(c + 1) * NPC][perms[c]]
        gm = np.zeros((NPAD, G), np.float32)
        gm[np.arange(NPC), bperm] = 1.0
        gmb = gm.reshape(NBLK, 128, G)
        g1h.append(np.ascontiguousarray(
            gmb.transpose(1, 0, 2).reshape(128, NBLK * G)))
        g1ht.append(np.ascontiguousarray(gm.T))
    # global one-hot over table layout [G, TROWS] (same for all cores)
    import ml_dtypes
    g1ht8 = np.concatenate(g1ht, axis=1).astype(ml_dtypes.bfloat16)
    assert g1ht8.shape == (G, N_CORES * NPAD)
    # per-shard graph runs over permuted+padded columns (same on all cores)
    segs8 = []
    for c in range(N_CORES):
        bp = batch[c * NPC:(c + 1) * NPC][perms[c]]
        bp_full = np.concatenate([bp, np.full(NPAD - NPC, bp[-1], bp.dtype)])
        bounds = ([0] + list(np.flatnonzero(np.diff(bp_full)) + 1) + [NPAD])
        segs8.append([(int(bounds[k]), int(bounds[k + 1]),
                       int(bp_full[bounds[k]]))
                      for k in range(len(bounds) - 1)])

    # x transposed + permuted + padded
    xT = []
    for c in range(N_CORES):
        xp = np.zeros((NPAD, x.shape[1]), np.float32)
        xp[:NPC] = x[c * NPC:(c + 1) * NPC][perms[c]]
        xT.append(np.ascontiguousarray(xp.T))

    # ---- weights
    in_W = np.asarray(inputs["in_W"], np.float32)
    in_b = np.asarray(inputs["in_b"], np.float32)
    Wg = np.asarray(inputs["Wg"], np.float32)
    att_src = np.asarray(inputs["att_src"], np.float32)
    att_dst = np.asarray(inputs["att_dst"], np.float32)
    gat_b = np.asarray(inputs["gat_b"], np.float32)
    gn_w = np.asarray(inputs["gn_w"], np.float32)
    gn_b = np.asarray(inputs["gn_b"], np.float32)
    gn_s = np.asarray(inputs["gn_s"], np.float32)

    W_z = np.zeros((L, HID, HID), np.float32)
    W_ed = np.zeros((L, HID, H), np.float32)
    Tinv_bd = np.zeros((L, HID, HID), np.float32)
    for l in range(L):
        for h in range(H):
            a = att_src[l, h]
            rng = np.random.default_rng(1234 + l * 16 + h)
            M = np.concatenate([a[:, None],
                                rng.standard_normal((DH, DH - 1))], 1)
            q, _ = np.linalg.qr(M)
            T = np.concatenate([a[:, None], q[:, 1:]], 1)  # [16,16]
            Ti = np.linalg.inv(T)
            sl = slice(h * DH, (h + 1) * DH)
            W_z[l][:, sl] = Wg[l][:, sl] @ T
            Tinv_bd[l][sl, sl] = Ti
            W_ed[l][:, h] = Wg[l][:, sl] @ att_dst[l, h]

    cnt_recip = np.zeros(G, np.float32)
    nz = cnt > 0
    cnt_recip[nz] = (1.0 / cnt[nz]).astype(np.float32)

    s = gn_s
    s2c = 2.0 * s - s * s

    consts = dict(
        inw=in_W,
        inb=in_b.reshape(HID, 1),
        wz=W_z, wed=W_ed, tinv=Tinv_bd,
        gatb=np.ascontiguousarray(gat_b.T),
        gnw=np.ascontiguousarray(gn_w.T),
        gnb=np.ascontiguousarray(gn_b.T),
        gns=np.ascontiguousarray(s.T),
        gns2c=np.ascontiguousarray(s2c.T),
        cntr=np.tile(cnt_recip[None, :], (HID, 1)),
        ident=np.eye(HID, dtype=np.float32),
        padrow=_padrow(),
    )

    return dict(d=d, units=units, d_i=d_i, idx_w=idx_w,
                idx_arrs=idx_arrs, g1h=g1h, g1ht=g1ht, g1ht8=g1ht8,
                segs8=segs8, xT=xT,
                perms=perms, inv_perms=inv_perms, consts=consts,
                batch=batch)


# ------------------------------------------------- numpy device emulation
# Mirrors the v2 device program (layouts, pads, bf16 rounding points) so host
# logic can be validated without a compile.


def _bf16(a):
    import ml_dtypes
    return a.astype(ml_dtypes.bfloat16).astype(np.float32)


def _coefs(tot, C, lidx, G):
    """tot: [HID, 2G] global stats; returns c1, c0 [HID, G] (f32)."""
    mean = tot[:, :G] * C["cntr"]
    ex2 = tot[:, G:] * C["cntr"]
    var = ex2 - C["gns2c"][:, lidx:lidx + 1] * mean * mean
    rstd = 1.0 / np.sqrt(var + EPS)
    c1 = C["gnw"][:, lidx:lidx + 1] * rstd
    c0 = C["gnb"][:, lidx:lidx + 1] - C["gns"][:, lidx:lidx + 1] * mean * c1
    return c1, c0


def _numpy_pipeline(prep, dbg=None):
    d = prep["d"]
    L, G = d["L"], d["G"]
    NPC, NBLK, NPAD, TROWS, TBASE = (
        d["NPC"], d["NBLK"], d["NPAD"], d["TROWS"], d["TBASE"])
    C = prep["consts"]
    units, d_i = prep["units"], prep["d_i"]
    g1ht8 = prep["g1ht8"]

    # input proj (per core, [128 f, NPAD n])
    hT = [C["inw"].T @ prep["xT"][c] + C["inb"] for c in range(N_CORES)]
    if dbg is not None:
        dbg["h0"] = [h.copy() for h in hT]

    stats = None          # [N_CORES, HID, 2G] partials of current pre-norm h
    for l in range(L):
        # ---- exchange: hb8 = bf16(h^T) of all shards; stats hi/lo packed
        hb8 = [_bf16(hT[c]) for c in range(N_CORES)]
        if l >= 1:
            tot = np.zeros((HID, 2 * G), np.float32)
            for c in range(N_CORES):
                hi = _bf16(stats[c])
                lo = _bf16(stats[c] - hi)
                tot += hi + lo
            c1, c0 = _coefs(tot, C, l - 1, G)
            if dbg is not None and l == 1:
                dbg["st1"] = tot.copy()
            table_coefs = (c1, c0)
            c1b, c0b = _bf16(c1), _bf16(c0)
            for c in range(N_CORES):
                p1 = c1 @ prep["g1ht"][c]
                p0 = c0 @ prep["g1ht"][c]
                hT[c] = hT[c] * p1 + p0
        else:
            table_coefs = None

        # ---- table rows
        wzb = _bf16(C["wz"][l])
        tbl = np.zeros((TROWS, HID), np.float32)
        if table_coefs is None:
            for c in range(N_CORES):
                tbl[c * NPAD:(c + 1) * NPAD] = _bf16(hb8[c].T @ wzb)
        else:
            for c in range(N_CORES):
                hn = _bf16(_bf16(hb8[c] * (c1b @ prep["g1ht"][c]))
                           + (c0b @ prep["g1ht"][c]))
                tbl[c * NPAD:(c + 1) * NPAD] = _bf16(hn.T @ wzb)
        tbl[TROWS - 1] = 0.0
        tbl[TROWS - 1, 0::DH] = PAD_ES
        if dbg is not None and l == 0:
            dbg["tbl0"] = tbl.copy()

        # ---- ed from own (f32) normalized h
        eds = [C["wed"][l].T @ hT[c] for c in range(N_CORES)]   # [H, NPAD]
        if dbg is not None and l == 0:
            dbg["ed0"] = [e.copy() for e in eds]

        # ---- edge phase per core
        new_stats = np.zeros((N_CORES, HID, 2 * G), np.float32)
        for c in range(N_CORES):
            idx = prep["idx_arrs"][c]
            col = 0
            for i in range(NBLK):
                di = int(d_i[i])
                w = idx[:16, col:col + di * 8]
                col += di * 8
                flat = w.T.reshape(-1)
                rows = tbl[flat.astype(np.int64) + TBASE]
                msg = rows.reshape(di, 128, HID).transpose(1, 0, 2)
                es = msg[:, :, 0::DH]                      # [128, di, H]
                ed = eds[c][:, i * 128:(i + 1) * 128].T    # [128, H]
                e = es + ed[:, None, :]
                e = np.where(e >= 0, e, NEG_SLOPE * e)
                exb = _bf16(np.exp(e))
                den = exb.sum(1)                           # [128, H]
                alph = _bf16(exb / den[:, None, :])
                mp = _bf16(msg * alph.repeat(DH, axis=2))
                agg = mp.sum(1)                            # [128 j, 128 f]
                attnT = C["tinv"][l].T @ agg.T             # [f', j]
                sl = slice(i * 128, (i + 1) * 128)
                xt = attnT + C["gatb"][:, l:l + 1]
                hT[c][:, sl] = xt + hT[c][:, sl]
                g1hb = prep["g1h"][c][:, i * G:(i + 1) * G]
                xb = hT[c][:, sl].T                        # [j, f]
                new_stats[c, :, :G] += (xb.T @ g1hb)
                new_stats[c, :, G:] += ((xb * xb).T @ g1hb)
            if dbg is not None and l == 0 and c == 0:
                dbg["x0"] = hT[0].copy()
        stats = new_stats

    # ---- final norm
    tot = np.zeros((HID, 2 * G), np.float32)
    for c in range(N_CORES):
        hi = _bf16(stats[c])
        lo = _bf16(stats[c] - hi)
        tot += hi + lo
    c1, c0 = _coefs(tot, C, L - 1, G)
    for c in range(N_CORES):
        p1 = c1 @ prep["g1ht"][c]
        p0 = c0 @ prep["g1ht"][c]
        hT[c] = hT[c] * p1 + p0

    # ---- output assembly
    N = d["N"]
    out = np.zeros((N, HID), np.float32)
    for c in range(N_CORES):
        out[c * NPC + prep["perms"][c]] = hT[c][:, :NPC].T
    return out


# ---------------------------------------------------------------- device


def _build_program(prep, timing_reps=None, dbg=True, ablate=()):
    import contextlib

    import concourse.tile as tile
    from concourse import bacc, mybir, library_config

    d = prep["d"]
    L, G = d["L"], d["G"]
    NPC, NBLK, NPAD, TROWS, TBASE, XW = (
        d["NPC"], d["NBLK"], d["NPAD"], d["TROWS"], d["TBASE"], d["XW"])
    units, d_i, idx_w = prep["units"], prep["d_i"], prep["idx_w"]
    DIMAX = int(d_i.max())
    F_IN = prep["xT"][0].shape[0]
    NCHUNK = (NPAD + 511) // 512
    AF = mybir.ActivationFunctionType
    ALU = mybir.AluOpType

    f32, bf16, i16 = mybir.dt.float32, mybir.dt.bfloat16, mybir.dt.int16

    nc = bacc.Bacc(None, target_bir_lowering=False, num_swdge_queues=4)

    def param(name, shape, dtype=f32, out=False):
        return nc.declare_dram_parameter(name, list(shape), dtype, isOutput=out)

    P = dict(
        xT=param("xT", [F_IN, NPAD]),
        idx=param("idx", [128, idx_w], i16),
        g1h=param("g1h", [128, NBLK * G]),
        g1ht=param("g1ht", [G, NBLK * 128]),
        g1ht8=param("g1ht8", [G, TROWS], bf16),
        inw=param("inw", [F_IN, HID]),
        inb=param("inb", [HID, 1]),
        wz=param("wz", [L, HID, HID]),
        wed=param("wed", [L, HID, H]),
        tinv=param("tinv", [L, HID, HID]),
        gatb=param("gatb", [HID, L]),
        gnw=param("gnw", [HID, L]),
        gnb=param("gnb", [HID, L]),
        gns=param("gns", [HID, L]),
        gns2c=param("gns2c", [HID, L]),
        cntr=param("cntr", [HID, G]),
        ident=param("ident", [HID, HID]),
        padrow=param("padrow", [1, HID], bf16),
        out=param("out", [NPC, HID], out=True),
        dbg_h0=param("dbg_h0", [HID, NPAD], out=True),
        dbg_x0=param("dbg_x0", [HID, NPAD], out=True),
        dbg_tbl=param("dbg_tbl", [TROWS, HID], bf16, out=True),
        dbg_ed=param("dbg_ed", [128, NBLK * H], out=True),
        dbg_st=param("dbg_st", [HID, 2 * G], out=True),
    )

    qctr = [0]

    def next_q():
        if QUEUE_SEQ is not None:
            q = QUEUE_SEQ[qctr[0] % len(QUEUE_SEQ)]
        else:
            q = qctr[0] % 4
        qctr[0] += 1
        return q

    with tile.TileContext(nc) as tc:
        est = contextlib.ExitStack()
        singles = est.enter_context(tc.tile_pool(name="singles", bufs=1))
        msgpA = est.enter_context(tc.tile_pool(name="msgA", bufs=3))
        msgpB = est.enter_context(tc.tile_pool(name="msgB", bufs=8))
        ep = est.enter_context(tc.tile_pool(name="etile", bufs=5))
        blkp = est.enter_context(tc.tile_pool(name="blk", bufs=5))
        stag = est.enter_context(tc.tile_pool(name="stag", bufs=2))
        cons = est.enter_context(tc.tile_pool(name="cons", bufs=2))
        psA = est.enter_context(tc.tile_pool(name="psA", bufs=3, space="PSUM"))
        psnp = est.enter_context(tc.tile_pool(name="psnp", bufs=2, space="PSUM"))
        pssm = est.enter_context(tc.tile_pool(name="pssm", bufs=2, space="PSUM"))
        psacc = est.enter_context(tc.tile_pool(name="psacc", bufs=1, space="PSUM"))
        dram = est.enter_context(tc.tile_pool(name="dram", bufs=1, space="DRAM"))

        nc.gpsimd.load_library(library_config.mlp)

        # ---------------- constants
        def load(t, src):
            nc.sync.dma_start(out=t, in_=src)
            return t

        ident_f = load(singles.tile([HID, HID], f32, name="idf"), P["ident"][:])
        ident_b = singles.tile([HID, HID], bf16, name="idb")
        nc.vector.tensor_copy(out=ident_b, in_=ident_f)
        idx_sb = load(singles.tile([128, idx_w], i16, name="idxs"), P["idx"][:])
        g1h_sb = load(singles.tile([128, NBLK * G], f32, name="g1h"), P["g1h"][:])
        inw_sb = load(singles.tile([F_IN, HID], f32, name="inw"), P["inw"][:])
        inb_sb = load(singles.tile([HID, 1], f32, name="inb"), P["inb"][:])
        wz_sb = [load(singles.tile([HID, HID], f32, name=f"wz{l}"), P["wz"][l])
                 for l in range(L)]
        wzb_sb = []
        for l in range(L):
            t = singles.tile([HID, HID], bf16, name=f"wzb{l}")
            nc.vector.tensor_copy(out=t, in_=wz_sb[l])
            wzb_sb.append(t)
        wed_sb = [load(singles.tile([HID, H], f32, name=f"wed{l}"), P["wed"][l])
                  for l in range(L)]
        tinv_sb = [load(singles.tile([HID, HID], f32, name=f"ti{l}"), P["tinv"][l])
                   for l in range(L)]
        gatb_sb = load(singles.tile([HID, L], f32, name="gatb"), P["gatb"][:])
        gnw_sb = load(singles.tile([HID, L], f32, name="gnw"), P["gnw"][:])
        gnb_sb = load(singles.tile([HID, L], f32, name="gnb"), P["gnb"][:])
        gns_sb = load(singles.tile([HID, L], f32, name="gns"), P["gns"][:])
        gns2c_sb = load(singles.tile([HID, L], f32, name="gns2c"), P["gns2c"][:])
        cntr_sb = load(singles.tile([HID, G], f32, name="cntr"), P["cntr"][:])
        eps_sb = singles.tile([HID, 1], f32, name="eps")
        nc.vector.memset(eps_sb, EPS)
        ones1 = singles.tile([1, HID], bf16, name="ones1")
        nc.vector.memset(ones1, 1.0)

        h_a = singles.tile([HID, NPAD], f32, name="h_a")
        ed_all = singles.tile([128, NBLK * H], f32, name="ed_all")

        hx_in = [dram.tile([128, XW], bf16, name=f"hxi{l}") for l in range(L)]
        hx_out = [dram.tile([N_CORES * 128, XW], bf16, addr_space="Shared",
                            name=f"hxo{l}") for l in range(L)]
        sx_in = dram.tile([128, 4 * G], bf16, name="sxi")
        sx_out = dram.tile([N_CORES * 128, 4 * G], bf16, addr_space="Shared",
                           name="sxo")
        tblr = [dram.tile([TROWS, HID], bf16, name=f"tbl{l}") for l in range(L)]

        # ---------------- helpers

        def inproj():
            for k in range(NCHUNK):
                c0, c1_ = k * 512, min((k + 1) * 512, NPAD)
                w = c1_ - c0
                xt = stag.tile([F_IN, 512], f32, name="xchunk")
                nc.sync.dma_start(out=xt[:, :w], in_=P["xT"][:, c0:c1_])
                ps = psA.tile([HID, 512], f32, name="psbig")
                nc.tensor.matmul(out=ps[:, :w], lhsT=inw_sb, rhs=xt[:, :w],
                                 start=True, stop=True)
                nc.scalar.activation(out=h_a[:, c0:c1_], in_=ps[:, :w],
                                     func=AF.Identity, bias=inb_sb[:, 0:1])

        def producer(l):
            """bf16 copies of h_a -> hx_in[l][:, :NPAD]."""
            for k in range(NCHUNK):
                c0, c1_ = k * 512, min((k + 1) * 512, NPAD)
                w = c1_ - c0
                hxb = stag.tile([128, 512], bf16, name="hxb")
                nc.scalar.activation(out=hxb[:, :w], in_=h_a[:, c0:c1_],
                                     func=AF.Copy)
                nc.sync.dma_start(out=hx_in[l][:, c0:c1_], in_=hxb[:, :w])

        def exchange(l):
            if timing_reps is None:
                nc.gpsimd.collective_compute(
                    "AllGather", mybir.AluOpType.bypass,
                    replica_groups=[list(range(N_CORES))],
                    ins=[hx_in[l].opt()], outs=[hx_out[l].opt()])
            else:
                nc.sync.dma_start(out=hx_out[l][:128, :], in_=hx_in[l][:])

        def exchange_final():
            if timing_reps is None:
                nc.gpsimd.collective_compute(
                    "AllGather", mybir.AluOpType.bypass,
                    replica_groups=[list(range(N_CORES))],
                    ins=[sx_in.opt()], outs=[sx_out.opt()])
            else:
                nc.sync.dma_start(out=sx_out[:128, :], in_=sx_in[:])

        def stats_coefs(stats_src, lidx):
            """Reconstruct global stats from 8 hi/lo shards; return c1T, c0T
            ([G, HID] f32 SBUF) and the [HID, G] c1/c0 tiles."""
            sacc = blkp.tile([128, 2 * G], f32, name="sacc")
            shl = cons.tile([128, N_CORES, 4 * G], bf16, name="shl")
            nc.sync.dma_start(
                out=shl, in_=stats_src().rearrange("(c p) w -> p c w", p=128))
            s64 = blkp.tile([128, 4 * G], f32, name="s64")
            nc.vector.tensor_reduce(
                out=s64, in_=shl.rearrange("p c w -> p w c"),
                axis=mybir.AxisListType.X, op=ALU.add)
            nc.vector.tensor_tensor(out=sacc, in0=s64[:, :2 * G],
                                    in1=s64[:, 2 * G:], op=ALU.add)
            if dbg and lidx == 0:
                nc.sync.dma_start(out=P["dbg_st"][:], in_=sacc)
            mean = blkp.tile([128, G], f32, name="mean")
            nc.vector.tensor_tensor(out=mean, in0=sacc[:, :G], in1=cntr_sb,
                                    op=ALU.mult)
            ex2 = blkp.tile([128, G], f32, name="ex2")
            nc.vector.tensor_tensor(out=ex2, in0=sacc[:, G:], in1=cntr_sb,
                                    op=ALU.mult)
            m2 = blkp.tile([128, G], f32, name="m2")
            nc.vector.tensor_tensor(out=m2, in0=mean, in1=mean, op=ALU.mult)
            nc.vector.tensor_tensor(
                out=m2, in0=m2,
                in1=gns2c_sb[:, lidx:lidx + 1].to_broadcast([HID, G]),
                op=ALU.mult)
            var = blkp.tile([128, G], f32, name="var")
            nc.vector.tensor_tensor(out=var, in0=ex2, in1=m2, op=ALU.subtract)
            nc.scalar.activation(out=var, in_=var, func=AF.Sqrt,
                                 bias=eps_sb[:, 0:1])
            rstd = blkp.tile([128, G], f32, name="rstd")
            nc.vector.reciprocal(out=rstd, in_=var)
            c1 = blkp.tile([128, G], f32, name="c1")
            nc.vector.tensor_tensor(
                out=c1, in0=rstd,
                in1=gnw_sb[:, lidx:lidx + 1].to_broadcast([HID, G]),
                op=ALU.mult)
            c0t = blkp.tile([128, G], f32, name="c0t")
            nc.vector.tensor_tensor(out=c0t, in0=mean, in1=c1, op=ALU.mult)
            nc.vector.tensor_tensor(
                out=c0t, in0=c0t,
                in1=gns_sb[:, lidx:lidx + 1].to_broadcast([HID, G]),
                op=ALU.mult)
            c0 = blkp.tile([128, G], f32, name="c0")
            nc.vector.tensor_tensor(
                out=c0, in0=gnb_sb[:, lidx:lidx + 1].to_broadcast([HID, G]),
                in1=c0t, op=ALU.subtract)
            pc = pssm.tile([128, HID], f32, name="pssm")
            nc.tensor.matmul(out=pc[:G, :], lhsT=c1, rhs=ident_f,
                             start=True, stop=True)
            c1T = blkp.tile([G, HID], f32, name="c1T")
            nc.vector.tensor_copy(out=c1T, in_=pc[:G, :])
            c1Tb = blkp.tile([G, HID], bf16, name="c1Tb")
            nc.vector.tensor_copy(out=c1Tb, in_=pc[:G, :])
            pc2 = pssm.tile([128, HID], f32, name="pssm")
            nc.tensor.matmul(out=pc2[:G, :], lhsT=c0, rhs=ident_f,
                             start=True, stop=True)
            c0T = blkp.tile([G, HID], f32, name="c0T")
            nc.vector.tensor_copy(out=c0T, in_=pc2[:G, :])
            c0Tb = blkp.tile([G, HID], bf16, name="c0Tb")
            nc.vector.tensor_copy(out=c0Tb, in_=pc2[:G, :])
            return dict(c1T=c1T, c0T=c0T, c1Tb=c1Tb, c0Tb=c0Tb)

        CW = 2048

        def table_pass(l, coefs):
            """Build tblr[l] from hx_out[l] (all 8 shards). DMAs move
            CW-column chunks; compute runs in 512-col slices."""
            for c in range(N_CORES):
                r0 = c * 128
                for kk in range((NPAD + CW - 1) // CW):
                    b0, b1 = kk * CW, min((kk + 1) * CW, NPAD)
                    wa = b1 - b0
                    hb = cons.tile([128, CW], bf16, name="hb")
                    nc.sync.dma_start(out=hb[:, :wa],
                                      in_=hx_out[l][r0:r0 + 128, b0:b1])
                    if coefs is not None:
                        g1 = cons.tile([G, CW], bf16, name="g1c")
                        nc.sync.dma_start(
                            out=g1[:, :wa],
                            in_=P["g1ht8"][:, c * NPAD + b0:c * NPAD + b1])
                    rows = cons.tile([128, CW], bf16, name="rows")
                    for k2 in range((wa + 511) // 512):
                        o = k2 * 512
                        w = min(512, wa - o)
                        nb = w // 128
                        if coefs is not None:
                            pp1 = psA.tile([128, 512], f32, name="psbig")
                            nc.tensor.matmul(out=pp1[:, :w],
                                             lhsT=coefs["c1Tb"],
                                             rhs=g1[:, o:o + w],
                                             start=True, stop=True)
                            pp0 = psA.tile([128, 512], f32, name="psbig")
                            nc.tensor.matmul(out=pp0[:, :w],
                                             lhsT=coefs["c0Tb"],
                                             rhs=g1[:, o:o + w],
                                             start=True, stop=True)
                            hn = cons.tile([128, 512], bf16, name="hn")
                            nc.vector.tensor_tensor(
                                out=hn[:, :w], in0=hb[:, o:o + w],
                                in1=pp1[:, :w], op=ALU.mult)
                            nc.vector.tensor_tensor(
                                out=hn[:, :w], in0=hn[:, :w],
                                in1=pp0[:, :w], op=ALU.add)
                            hns = hn[:, :w]
                        else:
                            hns = hb[:, o:o + w]
                        zp = psA.tile([128, 512], f32, name="psbig")
                        for j in range(nb):
                            nc.tensor.matmul(
                                out=zp[:, j * 128:(j + 1) * 128],
                                lhsT=hns[:, j * 128:(j + 1) * 128],
                                rhs=wzb_sb[l], start=True, stop=True)
                        nc.scalar.activation(out=rows[:, o:o + w],
                                             in_=zp[:, :w], func=AF.Copy)
                    rr = c * NPAD + b0
                    nc.sync.dma_start(
                        out=tblr[l][rr:rr + wa, :].rearrange(
                            "(b p) f -> p b f", p=128),
                        in_=rows[:, :wa].rearrange("p (b f) -> p b f", f=HID))
            nc.sync.dma_start(out=tblr[l][TROWS - 1:TROWS, :],
                              in_=P["padrow"][:])

        def norm_own(lidx, coefs):
            """f32 GraphNorm of h_a in place (own shard), 512-col chunks."""
            c1T, c0T = coefs["c1T"], coefs["c0T"]
            for k in range(NCHUNK):
                c0_, c1_ = k * 512, min((k + 1) * 512, NPAD)
                w = c1_ - c0_
                g1htb = stag.tile([G, 512], f32, name="g1htb")
                nc.sync.dma_start(out=g1htb[:, :w],
                                  in_=P["g1ht"][:, c0_:c1_])
                p1 = psA.tile([128, 512], f32, name="psbig")
                nc.tensor.matmul(out=p1[:, :w], lhsT=c1T, rhs=g1htb[:, :w],
                                 start=True, stop=True)
                p0 = psA.tile([128, 512], f32, name="psbig")
                nc.tensor.matmul(out=p0[:, :w], lhsT=c0T, rhs=g1htb[:, :w],
                                 start=True, stop=True)
                sl = slice(c0_, c1_)
                nc.vector.tensor_tensor(out=h_a[:, sl], in0=h_a[:, sl],
                                        in1=p1[:, :w], op=ALU.mult)
                nc.vector.tensor_tensor(out=h_a[:, sl], in0=h_a[:, sl],
                                        in1=p0[:, :w], op=ALU.add)

        def ed_pass(l):
            # ed_all[j, i*H+h] = sum_f h_a[f, i*128+j] * wed[f, h]
            for k in range(NCHUNK):
                c0_, c1_ = k * 512, min((k + 1) * 512, NPAD)
                nbb = (c1_ - c0_) // 128
                psd = pssm.tile([128, HID], f32, name="pssm")
                for bb in range(nbb):
                    i = (c0_ + bb * 128) // 128
                    nc.tensor.matmul(
                        out=psd[:, bb * H:(bb + 1) * H],
                        lhsT=h_a[:, i * 128:(i + 1) * 128],
                        rhs=wed_sb[l], start=True, stop=True)
                nc.vector.tensor_copy(
                    out=ed_all[:, (c0_ // 128) * H:(c1_ // 128) * H],
                    in_=psd[:, :nbb * H])

        # ---------------- edge phase (3-stage software pipeline over blocks)
        stats12 = psacc.tile([HID, 2 * G], f32, name="sab")
        stats1 = stats12[:, :G]
        stats2 = stats12[:, G:]

        def edge_phase(l):
            ucol = {}
            col = 0
            for (i, s0, ncl) in units:
                ucol[(i, s0)] = col
                col += ncl * 8
            DMID = min(20, DIMAX)
            state = {}

            def stage1(i):  # gather + attention scalars
                di = int(d_i[i])
                if di > DMID:
                    msg = msgpA.tile([128, DIMAX, HID], bf16, name="msgA")
                else:
                    msg = msgpB.tile([128, DMID, HID], bf16, name="msgB")
                s0 = 0
                while s0 < di:
                    ncl = min(8, di - s0)
                    c0 = ucol[(i, s0)]
                    nidx = ncl * 128
                    if "gather" not in ablate:
                        nc.gpsimd.dma_gather(
                            out_ap=msg[:, s0:s0 + ncl, :],
                            in_ap=tblr[l][TBASE:, :],
                            idxs_ap=idx_sb[:, c0:c0 + ncl * 8],
                            num_idxs=nidx, num_idxs_reg=nidx, elem_size=HID,
                            queue_num=next_q())
                    else:
                        nc.vector.memset(msg[:, s0:s0 + ncl, :], 0.25)
                    s0 += ncl
                e_t = ep.tile([128, DIMAX, H], f32, name="e_t")
                nc.vector.tensor_tensor(
                    out=e_t[:, :di, :],
                    in0=msg[:, :di, 0:HID:DH],
                    in1=ed_all[:, i * H:(i + 1) * H].unsqueeze(1)
                        .to_broadcast([128, di, H]),
                    op=ALU.add)
                # leaky_relu(x) = (1+a)/2*x + (1-a)/2*|x|
                ab_t = ep.tile([128, DIMAX, H], f32, name="ab_t")
                nc.scalar.activation(out=ab_t[:, :di, :], in_=e_t[:, :di, :],
                                     func=AF.Abs,
                                     scale=(1.0 - NEG_SLOPE) / 2.0)
                nc.vector.tensor_scalar_mul(
                    e_t[:, :di, :], e_t[:, :di, :], (1.0 + NEG_SLOPE) / 2.0)
                nc.vector.tensor_tensor(out=e_t[:, :di, :], in0=e_t[:, :di, :],
                                        in1=ab_t[:, :di, :], op=ALU.add)
                exb = ep.tile([128, DIMAX, H], bf16, name="exb")
                nc.scalar.activation(out=exb[:, :di, :], in_=e_t[:, :di, :],
                                     func=AF.Exp)
                den = blkp.tile([128, H], f32, name="den")
                nc.vector.tensor_reduce(
                    out=den, in_=exb[:, :di, :].rearrange("p a b -> p b a"),
                    axis=mybir.AxisListType.X, op=ALU.add)
                rec = blkp.tile([128, H], f32, name="rec")
                nc.vector.reciprocal(out=rec, in_=den)
                alph = ep.tile([128, DIMAX, H], bf16, name="alph")
                nc.vector.tensor_tensor(
                    out=alph[:, :di, :], in0=exb[:, :di, :],
                    in1=rec.unsqueeze(1).to_broadcast([128, di, H]),
                    op=ALU.mult)
                state[i] = (msg, alph)

            def stage2(i):  # weight + 4-slot-wide aggregate
                di = int(d_i[i])
                msg, alph = state[i]
                nc.vector.tensor_tensor(
                    out=msg[:, :di, :].rearrange("p a (b c) -> p a b c", b=H),
                    in0=msg[:, :di, :].rearrange("p a (b c) -> p a b c", b=H),
                    in1=alph[:, :di, :].unsqueeze(3)
                        .to_broadcast([128, di, H, DH]),
                    op=ALU.mult)
                # accumulate groups of 4 slots side by side; window m holds
                # sum over slots m, m+4, m+8, ... (combined in stage3)
                nps4 = psnp.tile([128, 4 * HID], f32, name="psnp")
                ng = (di + 3) // 4
                for g in range(ng):
                    wg = min(4, di - 4 * g)
                    nc.tensor.matmul(
                        out=nps4[:, :wg * HID], lhsT=ident_b,
                        rhs=msg[:, 4 * g:4 * g + wg, :].rearrange(
                            "p a b -> p (a b)"),
                        start=(g == 0), stop=(g == ng - 1),
                        skip_group_check=True)
                state[i] = (nps4, min(4, di))

            def stage3(i):  # window-sum + transpose + unrotate + residual
                nps4, W = state.pop(i)
                gat = blkp.tile([128, HID], f32, name="gat")
                nc.vector.tensor_reduce(
                    out=gat,
                    in_=nps4[:, :W * HID].rearrange("p (a b) -> p b a", a=W),
                    axis=mybir.AxisListType.X, op=ALU.add)
                pgt = pssm.tile([128, HID], f32, name="pssm")
                nc.tensor.matmul(out=pgt, lhsT=gat, rhs=ident_f,
                                 start=True, stop=True)
                aggs = blkp.tile([128, HID], f32, name="aggs")
                nc.scalar.activation(out=aggs, in_=pgt, func=AF.Copy)
                pat = pssm.tile([128, HID], f32, name="pssm")
                nc.tensor.matmul(out=pat, lhsT=tinv_sb[l], rhs=aggs,
                                 start=True, stop=True)
                xt = blkp.tile([128, HID], f32, name="xt")
                nc.scalar.activation(out=xt, in_=pat, func=AF.Identity,
                                     bias=gatb_sb[:, l:l + 1])
                sl = slice(i * 128, (i + 1) * 128)
                nc.vector.tensor_tensor(out=h_a[:, sl], in0=xt,
                                        in1=h_a[:, sl], op=ALU.add)
                if "stats" in ablate:
                    return
                pxb = pssm.tile([128, HID], f32, name="pssm")
                nc.tensor.matmul(out=pxb, lhsT=h_a[:, sl], rhs=ident_f,
                                 start=True, stop=True)
                xb = blkp.tile([128, HID], f32, name="xb")
                nc.vector.tensor_copy(out=xb, in_=pxb)
                sq = blkp.tile([128, HID], f32, name="sq")
                nc.scalar.activation(out=sq, in_=xb, func=AF.Square)
                nc.tensor.matmul(out=stats1, lhsT=xb,
                                 rhs=g1h_sb[:, i * G:(i + 1) * G],
                                 start=(i == 0), stop=(i == NBLK - 1),
                                 skip_group_check=True)
                nc.tensor.matmul(out=stats2, lhsT=sq,
                                 rhs=g1h_sb[:, i * G:(i + 1) * G],
                                 start=(i == 0), stop=(i == NBLK - 1),
                                 skip_group_check=True)

            for i in range(NBLK + 2):
                if i < NBLK:
                    stage1(i)
                if 1 <= i <= NBLK and "post" not in ablate:
                    stage2(i - 1)
                if 2 <= i <= NBLK + 1 and "post" not in ablate:
                    stage3(i - 2)
                if "post" in ablate and i < NBLK:
                    msg, alph = state.pop(i)
                    nc.vector.tensor_copy(
                        out=h_a[:, i * 128 + 0:i * 128 + 8],
                        in_=alph[:, 0, :])

            # pack stats as hi/lo bf16 into the next exchange payload
            stl = stag.tile([HID, 2 * G], f32, name="stl")
            if "post" in ablate or "stats" in ablate:
                nc.vector.memset(stl, 1.0)
            else:
                nc.vector.tensor_copy(out=stl, in_=stats12)
            hilo = stag.tile([HID, 4 * G], bf16, name="hilo")
            nc.scalar.activation(out=hilo[:, :2 * G], in_=stl, func=AF.Copy)
            nc.vector.tensor_tensor(out=hilo[:, 2 * G:], in0=stl,
                                    in1=hilo[:, :2 * G], op=ALU.subtract)
            dst_ap = (hx_in[l + 1][:, NPAD:NPAD + 4 * G] if l < L - 1
                      else sx_in[:, :])
            nc.sync.dma_start(out=dst_ap, in_=hilo)

        def output_pass():
            for k in range(NCHUNK):
                c0_, c1_ = k * 512, min((k + 1) * 512, NPAD)
                r1 = min(c1_, NPC)
                if r1 <= c0_:
                    break
                w = c1_ - c0_
                nbb = w // 128
                po = psA.tile([128, 512], f32, name="psbig")
                for bb in range(nbb):
                    nc.tensor.matmul(
                        out=po[:, bb * 128:(bb + 1) * 128],
                        lhsT=h_a[:, c0_ + bb * 128:c0_ + (bb + 1) * 128],
                        rhs=ident_f, start=True, stop=True)
                rows = stag.tile([128, 512], f32, name="orow")
                nc.vector.tensor_copy(out=rows[:, :w], in_=po[:, :w])
                if r1 == c1_:
                    nc.sync.dma_start(
                        out=P["out"][c0_:c1_, :].rearrange(
                            "(b p) f -> p b f", p=128),
                        in_=rows[:, :w].rearrange("p (b f) -> p b f", f=HID))
                else:
                    # tail: whole 128-blocks then the partial block
                    full = ((r1 - c0_) // 128) * 128
                    if full:
                        nc.sync.dma_start(
                            out=P["out"][c0_:c0_ + full, :].rearrange(
                                "(b p) f -> p b f", p=128),
                            in_=rows[:, :full].rearrange(
                                "p (b f) -> p b f", f=HID))
                    part = r1 - c0_ - full
                    if part:
                        nc.sync.dma_start(
                            out=P["out"][c0_ + full:r1, :],
                            in_=rows[:part, full:full + 128])

        # ---------------- program
        loop_ctx = tc.For_i(0, timing_reps) if timing_reps else None
        if loop_ctx:
            loop_ctx.__enter__()
        inproj()
        if dbg:
            nc.sync.dma_start(out=P["dbg_h0"][:], in_=h_a)
        for l in range(L):
            producer(l)
            exchange(l)
            if l >= 1:
                coefs = stats_coefs(
                    lambda _l=l: hx_out[_l][:, NPAD:NPAD + 4 * G], l - 1)
            else:
                coefs = None
            table_pass(l, coefs)
            if coefs is not None:
                norm_own(l - 1, coefs)
            ed_pass(l)
            if l == 0 and dbg:
                nc.sync.dma_start(out=P["dbg_tbl"][:], in_=tblr[0][:])
                nc.sync.dma_start(out=P["dbg_ed"][:], in_=ed_all)
            edge_phase(l)
            if l == 0 and dbg:
                nc.sync.dma_start(out=P["dbg_x0"][:], in_=h_a)
        exchange_final()
        fcoefs = stats_coefs(lambda: sx_out[:, :], L - 1)
        norm_own(L - 1, fcoefs)
        output_pass()
        if loop_ctx:
            loop_ctx.__exit__(None, None, None)

        est.close()

    nc.compile()
    return nc


def _make_inmaps(prep):
    C = prep["consts"]
    maps = []
    for c in range(N_CORES):
        m = dict(
            xT=prep["xT"][c],
            idx=prep["idx_arrs"][c],
            g1h=prep["g1h"][c],
            g1ht=prep["g1ht"][c],
            g1ht8=prep["g1ht8"],
            inw=C["inw"], inb=C["inb"], wz=C["wz"], wed=C["wed"],
            tinv=C["tinv"], gatb=C["gatb"], gnw=C["gnw"], gnb=C["gnb"],
            gns=C["gns"], gns2c=C["gns2c"], cntr=C["cntr"], ident=C["ident"],
            padrow=C["padrow"],
        )
        maps.append(m)
    return maps


def _assemble(prep, results):
    d = prep["d"]
    NPC = d["NPC"]
    out = np.zeros((d["N"], HID), np.float32)
    for c in range(N_CORES):
        out[c * NPC + prep["perms"][c]] = results[c]["out"]
    return out


def _run(inputs, cfg):
    from concourse.bass_utils import run_bass_kernel_spmd
    prep = _preprocess(inputs, cfg)
    nc = _build_program(prep)
    res = run_bass_kernel_spmd(nc, _make_inmaps(prep),
                               core_ids=list(range(N_CORES)))
    return _assemble(prep, res.results)


def kernel(**inputs):
    return _run(inputs, FULL_CFG)
